# revision 93
# baseline (speedup 1.0000x reference)
"""KMeans vq_codebook kernel for 8 trn2 NeuronCores.

Strategy (data-parallel over N, per sharding hint). Per core (NS=32768
rows, 256 row-tiles of 128, K=256 centers, D=128), groups of 4 tiles:

  PE:   per tile, a rank-1 bias matmul (ones[1,128] x -csq[1,K],
        start=True) then the main fp16 matmul xh_tile[128d,128n] x
        ch[128d,K] accumulate on top -> PSUM holds h = 2x@c.T - |c|^2
        (f32) directly. Per tile PAIR: conf[16,K] += yoh8_pair.T @
        onehot_pair as an fp8e4 DoubleRow matmul (2 contraction rows per
        partition; classes padded 10->16 to satisfy the dual-fp8
        ldweights ISA restriction), into 2 interleaved PSUM accumulators.
  Act:  batched copy PSUM f32 -> SBUF fp16 downcast of h.
  DVE:  pairwise-max tree (fp16 tensor_tensor at the 2x perf mode) +
        reduce -> hmax[:, j] f32; plus 3 of every 8 onehot tiles.
  Pool: onehot = (h == hmax) via per-tile tensor_scalar (per-partition
        f32 scalar; fp8e4 output) for 5 of every 8 tiles.
  Host: loss = sum(x*x) (f64) - sum(hmax); conf summed over cores and
        accumulators -> acc = sum_k max_c conf / N.

Conf matmuls are emitted CONF_LAG groups late so the PE never waits on
the Act->DVE->Pool onehot chain. Precision: x and 2c quantized to fp16
(single matmul), h stored fp16. Measured against the f64 reference:
loss_rel ~1.3e-6, acc_rel ~2e-3 (fp16 ties in is_equal slightly inflate
conf; well within the 2e-2 gate).
"""

import sys

sys.path.insert(0, "/opt/trn_rl_repo")

import numpy as np

import concourse.bass as bass
import concourse.mybir as mybir
import concourse.tile as tile
from concourse.bass_utils import run_bass_kernel_spmd

N_FULL = 262144
D = 128
K = 256
NUM_CORES = 8
NS = N_FULL // NUM_CORES  # 32768 rows per core
NUM_GT_CLASSES = 10
NGC = NUM_GT_CLASSES

F32 = mybir.dt.float32
F16 = mybir.dt.float16
F8 = mybir.dt.float8e4
NGP = 16  # classes padded to 16 (fp8 DoubleRow ldweights needs M=16)

_CACHE = {}
TRACE = False  # set True (e.g. from test.py) to capture an NTFF profile
LAST = {}  # holds the last BassKernelResults when TRACE is on

GROUP = 4  # tiles per PSUM group (2 banks; x3 bufs + 2 conf banks = 8/8)
SUPER = 10  # tiles per x supertile DMA
# Engine schedule knobs, pattern period over groups:
#   slot in B_SLOTS    -> type-B group: subtract on DVE (no PE bias mm)
#   slot in EDVE_SLOTS -> onehot entirely on DVE (otherwise split by EDVE_COLS)
PERIOD = 10
B_SLOTS = ()
EDVE_SLOTS = ()
EDVE_COLS = 256  # (unused; kept for call compat)
NCONF = 2  # interleaved conf PSUM accumulators
CONF_DR = True  # even groups: Pool writes fp8 onehot, conf via fp8 DoubleRow
R_POOL = False  # run the final hmax reduce on the Pool engine (illegal on HW)
E_POOL_NTH = 5  # of every 8 tiles, this many run the onehot on Pool
CONF_LAG = 6  # groups of delay before a group's conf matmuls are emitted


def build_nc(ns=NS, for_sim=False, period=PERIOD, b_slots=B_SLOTS,
             edve_slots=EDVE_SLOTS, group=None, psbufs=3, hbufs=6,
             ohbufs=9, treebufs=6, xsbufs=3, super_=None, tree_stages=2,
             edve_cols=EDVE_COLS, nconf=NCONF, conf_dr=CONF_DR, r_pool=R_POOL,
             e_pool_nth=E_POOL_NTH, conf_lag=CONF_LAG, nwarm=0):
    """Build the single-core Bass program (same program runs SPMD on 8 cores)."""
    global GROUP, SUPER
    if group is not None:
        GROUP = group
    if super_ is not None:
        SUPER = super_
    ntiles = ns // 128

    import concourse.bacc as bacc

    nc = bacc.Bacc("TRN2", target_bir_lowering=False, debug=bool(for_sim))

    xh_d = nc.declare_dram_parameter("xh", [D, ns], F16, isOutput=False)
    ch_d = nc.declare_dram_parameter("ch", [D, K], F16, isOutput=False)
    # ones1 (cols 0:128) and -csq (cols 128:128+K) packed into one small DMA
    onc_d = nc.declare_dram_parameter("onc", [1, 128 + K], F16, isOutput=False)
    csqb_d = nc.declare_dram_parameter("csqb", [D, K], F16, isOutput=False)
    npairs = ntiles // 2
    yoh_d = nc.declare_dram_parameter("yoh", [128, npairs, 2, NGP], F16, isOutput=False)
    yoh8_d = nc.declare_dram_parameter("yoh8", [128, npairs, 2, NGP], F8, isOutput=False)
    hmax_out = nc.declare_dram_parameter("hmax", [128, ntiles], F32, isOutput=True)
    conf_out = nc.declare_dram_parameter("conf", [NGP, nconf * K], F32, isOutput=True)

    # group schedule: list of (start_tile, size)
    groups = []
    t0 = 0
    while t0 < ntiles:
        gsz = min(GROUP, ntiles - t0)
        groups.append((t0, gsz))
        t0 += gsz
    need_yoh16 = (not conf_dr) or any(gsz % 2 for _, gsz in groups)
    need_csqb = bool(b_slots)

    with tile.TileContext(nc) as tc:
        with (
            tc.tile_pool(name="const", bufs=1) as constp,
            tc.tile_pool(name="xs", bufs=xsbufs) as xsp,
            tc.tile_pool(name="hb", bufs=hbufs) as hbp,
            tc.tile_pool(name="ohb", bufs=ohbufs) as ohbp,
            tc.tile_pool(name="tree", bufs=treebufs) as treep,
            tc.tile_pool(name="acc", bufs=1) as accp,
            tc.tile_pool(name="ps", bufs=psbufs, space=bass.MemorySpace.PSUM) as psp,
            tc.tile_pool(name="psconf", bufs=1, space=bass.MemorySpace.PSUM) as pscp,
        ):
            ch_t = constp.tile([D, K], F16, tag="ch")
            onc_t = constp.tile([1, 128 + K], F16, tag="onc")
            csqb_t = constp.tile([D, K], F16, tag="csqb")
            yoh_t = constp.tile([128, npairs, 2, NGP], F16, tag="yoh")
            yoh8_t = constp.tile([128, npairs, 2, NGP], F8, tag="yoh8")
            nc.sync.dma_start(onc_t[:], onc_d[:, :])
            nc.sync.dma_start(ch_t[:], ch_d[:, :])
            ones1_t = onc_t[:, 0:128]
            negcsq_t = onc_t[:, 128 : 128 + K]
            yoh_loaded = [False]

            def load_late_consts():
                # yoh/csqb aren't needed until the first conf matmul /
                # type-B group; keep them off the DMA queue at startup
                if not yoh_loaded[0]:
                    if need_csqb:
                        nc.sync.dma_start(csqb_t[:], csqb_d[:, :])
                    if need_yoh16:
                        nc.sync.dma_start(yoh_t[:], yoh_d[:, :, :, :])
                    if conf_dr:
                        nc.sync.dma_start(yoh8_t[:], yoh8_d[:, :, :, :])
                    yoh_loaded[0] = True

            hmax_acc = accp.tile([128, ntiles], F32, tag="hmax")
            conf_ps = []
            for a in range(nconf):
                conf_a = pscp.tile([NGP, K], F32, tag=f"conf{a}", name=f"conf_ps{a}")
                conf_ps.append(conf_a)

            # supertile x loads; the first supertile is split small so the
            # first matmuls start sooner
            super_starts = []
            s0 = 0
            first_chunk = min(4 * 128, ns)
            super_starts.append((0, first_chunk))
            s0 = first_chunk
            while s0 < ns:
                ssz = min(SUPER * 128, ns - s0)
                super_starts.append((s0, ssz))
                s0 += ssz
            tile_to_super = {}
            for si, (st, ssz) in enumerate(super_starts):
                for j in range(st // 128, (st + ssz) // 128):
                    tile_to_super[j] = si
            xtiles = {}  # super index -> tile

            def xsuper_for_tile(j):
                si = tile_to_super[j]
                if si not in xtiles:
                    st, ssz = super_starts[si]
                    xt = xsp.tile([D, SUPER * 128], F16, tag="xh",
                                  name=f"xh_s{si}")
                    nc.sync.dma_start(xt[:, :ssz], xh_d[:, st : st + ssz])
                    xtiles[si] = (xt, st)
                return xtiles[si]

            ngroups = len(groups)
            # Warm the PE p-state while the first x chunk is still in
            # flight: dummy rank-1 matmuls depending only on the tiny onc
            # DMA. By the time real matmuls run, the PE clock is ramped.
            if nwarm:
                warm = psp.tile([128, GROUP, K], F32, tag="g2", name="warm")
                for wi in range(nwarm):
                    nc.tensor.matmul(
                        warm[:, wi % GROUP, :], ones1_t, negcsq_t,
                        start=True, stop=True, skip_group_check=True,
                    )

            first_conf = [True] * nconf
            last_group_for = {a: max(g for g in range(ngroups) if g % nconf == a)
                              for a in range(nconf)}
            pending_conf = []  # deque of (g, t0, gsz, oh), emitted conf_lag late

            def emit_conf(pc):
                pg, pt0, pgsz, poh, is_dr = pc
                a = pg % nconf
                if is_dr:
                    assert pt0 % 2 == 0 and pgsz % 2 == 0
                    for s in range(pgsz // 2):
                        pr = pt0 // 2 + s
                        nc.tensor.matmul(
                            conf_ps[a][:],
                            yoh8_t[:, pr, :, :],
                            poh[:, 2 * s : 2 * s + 2, :],
                            start=first_conf[a],
                            stop=(pg == last_group_for[a] and s == pgsz // 2 - 1),
                            perf_mode=mybir.MatmulPerfMode.DoubleRow,
                            skip_group_check=True,
                        )
                        first_conf[a] = False
                else:
                    for q in range(pgsz):
                        j = pt0 + q
                        pr, s = j // 2, j % 2
                        nc.tensor.matmul(
                            conf_ps[a][:],
                            yoh_t[:, pr, s, :],
                            poh[:, q, :],
                            start=first_conf[a],
                            stop=(pg == last_group_for[a] and q == pgsz - 1),
                            skip_group_check=True,
                        )
                        first_conf[a] = False

            for g, (t0, gsz) in enumerate(groups):
                slot = g % period
                type_b = slot in b_slots
                e_on_dve = slot in edve_slots
                if type_b:
                    load_late_consts()

                gps = psp.tile([128, GROUP, K], F32, tag="g2")

                for q in range(gsz):
                    j = t0 + q
                    xt, s0 = xsuper_for_tile(j)
                    sl = slice(j * 128 - s0, (j + 1) * 128 - s0)
                    if type_b:
                        nc.tensor.matmul(
                            gps[:, q, :], xt[:, sl], ch_t[:], start=True, stop=True
                        )
                    else:
                        nc.tensor.matmul(
                            gps[:, q, :], ones1_t, negcsq_t,
                            start=True, stop=False,
                        )
                        nc.tensor.matmul(
                            gps[:, q, :], xt[:, sl], ch_t[:], start=False, stop=True
                        )

                # conf matmuls of an earlier group go here: their oh is
                # ready by now, so they don't block this group's main mms.
                # Taper the lag near the end so the tail doesn't serialize.
                if g == 1:
                    load_late_consts()
                target_lag = min(conf_lag, ngroups - 1 - g)
                while len(pending_conf) > target_lag:
                    emit_conf(pending_conf.pop(0))

                # Act: downcast PSUM f32 -> SBUF fp16
                hq = hbp.tile([128, GROUP, K], F16, tag="h")
                nc.scalar.copy(hq[:, :gsz, :], gps[:, :gsz, :])

                if type_b:
                    # subtract csq on DVE (h overwritten in place is not
                    # allowed; use a second buffer)
                    h2 = hbp.tile([128, GROUP, K], F16, tag="h2")
                    csq_b = csqb_t[:].unsqueeze(1).broadcast_to([128, gsz, K])
                    nc.vector.tensor_tensor(
                        h2[:, :gsz, :], hq[:, :gsz, :], csq_b,
                        mybir.AluOpType.subtract,
                    )
                    hcur = h2
                else:
                    hcur = hq

                # DVE: pairwise-max tree then reduce -> hmax f32
                tcur = hcur
                w = 256
                for s in range(tree_stages):
                    w //= 2
                    tn = treep.tile([128, GROUP, w], F16, tag=f"t{s}")
                    nc.vector.tensor_tensor(
                        tn[:, :gsz, :], tcur[:, :gsz, 0:w], tcur[:, :gsz, w : 2 * w],
                        mybir.AluOpType.max,
                    )
                    tcur = tn
                reng = nc.gpsimd if r_pool else nc.vector
                reng.tensor_reduce(
                    hmax_acc[:, t0 : t0 + gsz],
                    tcur[:, :gsz, :],
                    axis=mybir.AxisListType.X,
                    op=mybir.AluOpType.max,
                )

                # onehot = (h == hmax): per-tile tensor_scalar, balanced
                # between Pool and DVE. With conf_dr, ALL onehots are fp8
                # and every conf matmul runs fp8 DoubleRow (uniform PE mode).
                is_dr = conf_dr and gsz % 2 == 0
                oh = ohbp.tile([128, GROUP, K], F8 if is_dr else F16,
                               tag="oh8" if is_dr else "oh")
                for q in range(gsz):
                    j = t0 + q
                    eng = nc.gpsimd if (j % 8) < e_pool_nth else nc.vector
                    eng.tensor_scalar(
                        oh[:, q, :], hcur[:, q, :],
                        hmax_acc[:, j : j + 1], None,
                        mybir.AluOpType.is_equal,
                    )

                pending_conf.append((g, t0, gsz, oh, is_dr))

                # first half of hmax is final once tile 128 is reduced;
                # ship it early to shorten the drain tail
                if t0 + gsz == ntiles // 2:
                    nc.sync.dma_start(
                        hmax_out[:, : ntiles // 2],
                        hmax_acc[:, : ntiles // 2],
                    )

            for pc in pending_conf:
                emit_conf(pc)

            conf_sb = accp.tile([NGP, nconf * K], F32, tag="confsb")
            for a in range(nconf):
                nc.vector.tensor_copy(
                    conf_sb[:, a * K : (a + 1) * K], conf_ps[a][:]
                )
            nc.sync.dma_start(
                hmax_out[:, ntiles // 2 :], hmax_acc[:, ntiles // 2 :]
            )
            nc.sync.dma_start(conf_out[:, :], conf_sb[:])

    nc.compile()
    return nc


def kernel(x, y, centers):
    x = np.asarray(x, dtype=np.float32)
    y_np = np.asarray(y).astype(np.int64)
    centers = np.asarray(centers, dtype=np.float32)
    n = x.shape[0]
    assert n == N_FULL and x.shape[1] == D and centers.shape == (K, D)

    if "nc" not in _CACHE:
        _CACHE["nc"] = build_nc()
    nc = _CACHE["nc"]

    ntiles = NS // 128

    xt = np.ascontiguousarray(x.T)  # [128, N] f32
    xh = xt.astype(np.float16)
    ch = (np.ascontiguousarray(centers.T) * np.float32(2.0)).astype(np.float16)
    csq = np.sum(centers.astype(np.float64) ** 2, axis=1)
    csq16 = csq.astype(np.float16)
    onc = np.concatenate(
        [np.ones(128, dtype=np.float16), (-csq16).astype(np.float16)]
    )[None, :]  # [1, 128+K] fp16
    csqb = np.ascontiguousarray(np.broadcast_to(csq16[None, :], (D, K)))

    # One-hot labels, classes padded to NGP=16, laid out per tile:
    # yoh[p, 16*j + c] = (y[j*128+p] == c)
    import ml_dtypes

    y_cores = y_np.reshape(NUM_CORES, ntiles, 128)  # [core, tile j, p]
    oh = (y_cores[:, :, :, None] == np.arange(NGP)[None, None, None, :]).astype(
        np.float16
    )  # [core, j, p, c]
    yoh_all = np.ascontiguousarray(
        oh.transpose(0, 2, 1, 3).reshape(NUM_CORES, 128, ntiles * NGP)
    )
    yoh8_all = yoh_all.astype(ml_dtypes.float8_e4m3fn)

    in_maps = []
    for c in range(NUM_CORES):
        sl = slice(c * NS, (c + 1) * NS)
        in_maps.append(
            {
                "xh": np.ascontiguousarray(xh[:, sl]),
                "ch": ch,
                "onc": onc,
                "csqb": csqb,
                "yoh": yoh_all[c],
                "yoh8": yoh8_all[c],
            }
        )

    out = run_bass_kernel_spmd(nc, in_maps, list(range(NUM_CORES)), trace=TRACE)
    LAST["results"] = out
    res = out.results

    hmax_sum = 0.0
    conf = np.zeros((K, NGC), dtype=np.float64)
    for c in range(NUM_CORES):
        hmax_sum += float(np.asarray(res[c]["hmax"]).astype(np.float64).sum())
        cc = np.asarray(res[c]["conf"]).astype(np.float64)  # [16, nconf*K]
        conf += cc.reshape(NGP, -1, K).sum(axis=1).T[:, :NGC]  # [K, 10]

    x64 = x.astype(np.float64)
    x_sq_total = float(np.einsum("nd,nd->", x64, x64, optimize=True))
    loss = np.float32(x_sq_total - hmax_sum)

    correct_ct = conf.max(axis=1).sum()
    acc = np.float32(correct_ct / np.float32(n))
    return loss, acc
